# revision 28
# baseline (speedup 1.0000x reference)
"""Multi-head causal attention (B=2,T=2048,C=1024,H=16,Dh=64) on 8 trn2 cores.

Sharding: tensor-parallel over heads - core c owns heads (2c, 2c+1).

v3 dataflow (vs the v1 baseline; 196.4us -> 138.6us in the CoreSim
cost model, correctness verified on the 8-core trn2 fixture):
- att@v is "flipped": outputs live as [query, head_dim] so every att@v
  matmul has 65-column cost with all 128 PE partitions doing useful work
  (half the PE time of the [head_dim, query] orientation). The softmax
  denominator lands in psum column 64 via the ones-column of v, so the
  normalization is a per-partition reciprocal + tensor_scalar (no DRAM
  broadcast round-trip).
- v is computed token-major directly on the PE (lhsT = x chunk, rhs =
  Wv), so no transpose is ever needed in the strip stream. The
  normalized attention outputs ship to the reshard slots untransposed
  ([token, head_dim]) and are transposed on the receive side: chunks
  1/2 via PE-transposes (their windows overlap later collectives, and
  DMA-xbar transposes mutually exclude with in-flight collectives),
  chunk 3 via transposing DMA reads (nothing is in flight after the
  last collective).
- strips of both batches interleave (b0s0,b0s1,b1s0,b1s1,b0s2,b1s2,
  b0s3,b1s3) and the reshard is split into three all-to-alls over token
  ranges [0,1024), [1024,1536), [1536,2048) x both batches. All three
  collectives are emitted after the last strip (the Pool queue carries
  nothing else, so each launches as soon as its input slots land), and
  only the last one plus one small projection is exposed at the tail.
- within a strip the scores matmul for tile jt+1 is emitted before the
  att@v matmuls of tile jt, so the PE stays one tile ahead of the
  Activation engine and the exp stream (the attention-phase bottleneck,
  ~74us) never starves.
- QKV of both batches is emitted as "filler" pulled into the PE stream
  between attention tiles to cover the exp latency.
- DMA queues: SP and Act transfers overlap in the DMA fabric; the x
  stream + half the reshard traffic ride SP, the weight stream + the
  other half ride Act, and the Pool queue is reserved for collectives.
"""
import numpy as np
import ml_dtypes

import concourse.bass as bass
import concourse.mybir as mybir
import concourse.tile as tile
from concourse.bass_utils import run_bass_kernel_spmd
from concourse.masks import make_identity
from concourse.tile import add_dep_helper
from concourse.vector_clock import ScopedClock

BF16 = mybir.dt.bfloat16
F32 = mybir.dt.float32

B, T, C = 2, 2048, 1024
H, DH = 16, 64
NCORES = 8
HPC = 128  # head-columns per core (2 heads x 64)
NI = 512   # query-strip width
NJ = 128   # key-tile width
NSTRIP = T // NI          # 4 strips per batch
NJT = T // NJ             # 16 key tiles per batch
NCC = C // 128            # 8 contraction chunks
TSW = 512                 # token-chunk width for QKV compute
XLW = 256                 # token-chunk width for x loads
SCALE = DH ** -0.5


class TileContextP(tile.TileContext):
    """This walrus build caps sync waits at 1 per instruction (2 for
    EventSemaphore). Tile can emit more. Legalize by spilling excess waits
    onto same-engine nops emitted just before the instruction, and do the
    same for the kernel-tail drain."""

    def _commit_instruction(self, inst, lazy_reg_writes: bool = True):
        si = getattr(inst, "sync_info", None)
        if si is not None and si.on_wait:
            cap = 2 if isinstance(inst, mybir.InstEventSemaphore) else 1
            if len(si.on_wait) > cap:
                waits = list(si.on_wait)
                keep, spill = waits[:cap - 1] if cap > 1 else [], waits[cap - 1:]
                # keep the last wait on the inst, spill the rest
                spill, last = spill[:-1], spill[-1:]
                for w in spill:
                    nop = mybir.InstNoOp(
                        name=self.nc.get_next_instruction_name(),
                        engine=inst.engine,
                        sync_info=mybir.SyncInfo(on_wait=[w], on_update=[]),
                        bass_nofuse=True,
                    )
                    self._add_instruction(nop)
                si.on_wait = keep + last
        return super()._commit_instruction(inst, lazy_reg_writes)

    def _drain_and_barrier(self, tick_clock, wait_clock):
        probe = self.nc.sync.nop()
        wait_clock.add_sem_waits(
            probe.ins, ScopedClock({None: tick_clock.global_clock})
        )
        waits = list(probe.ins.sync_info.on_wait) if probe.ins.sync_info else []
        if probe.ins.sync_info:
            probe.ins.sync_info.on_wait = []
        for w in waits:
            n = self.nc.sync.nop()
            si = n.ins.sync_info
            if si is None:
                n.ins.sync_info = mybir.SyncInfo(on_wait=[w], on_update=[])
            else:
                si.on_wait = [w]
        self.nc.sync.drain()
        self.nc.all_engine_barrier()
        assert self.sems is not None
        popped = self.nc._tile_sem_poison_stack.pop()
        assert popped is self._sem_poison
        self.nc.clear_and_free_semaphores(list(self.sems.allocated().values()))
        self.nc.all_engine_barrier()


def build_nc():
    nc = bass.Bass()
    xT_h = nc.dram_tensor("xT", [B, C, T], BF16, kind="ExternalInput")
    wq_h = nc.dram_tensor("wq", [C, HPC], BF16, kind="ExternalInput")
    wk_h = nc.dram_tensor("wk", [C, HPC], BF16, kind="ExternalInput")
    wv_h = nc.dram_tensor("wv", [C, HPC], BF16, kind="ExternalInput")
    wp_h = nc.dram_tensor("wp", [C, C], BF16, kind="ExternalInput")
    mk_h = nc.dram_tensor("masks", [4, NJ, NI], BF16, kind="ExternalInput")
    y_h = nc.dram_tensor("y_out", [C, 512], F32, kind="ExternalOutput")
    # reshard buffers, slots are [tokens, head_dim(128)]
    # chunk 1: tokens [0,1024) of both batches; slot j<4 -> b0, j>=4 -> b1
    a1_in = nc.dram_tensor("a1_in", [NCORES, 256, 128], BF16)
    a1_out = nc.dram_tensor("a1_out", [NCORES, 256, 128], BF16)
    # chunk 2: tokens [1024,1536); chunk 3: tokens [1536,2048)
    a2_in = nc.dram_tensor("a2_in", [NCORES, 128, 128], BF16)
    a2_out = nc.dram_tensor("a2_out", [NCORES, 128, 128], BF16)
    a3_in = nc.dram_tensor("a3_in", [NCORES, 128, 128], BF16)
    a3_out = nc.dram_tensor("a3_out", [NCORES, 128, 128], BF16)

    with TileContextP(nc) as tc, \
         tc.tile_pool(name="singles", bufs=1) as singles, \
         tc.tile_pool(name="xtp", bufs=9) as xtp, \
         tc.tile_pool(name="qkp", bufs=2) as qkp, \
         tc.tile_pool(name="vaugp", bufs=2) as vaugp, \
         tc.tile_pool(name="weip", bufs=6) as weip, \
         tc.tile_pool(name="attnp", bufs=20) as attnp, \
         tc.tile_pool(name="rcpp", bufs=4) as rcpp, \
         tc.tile_pool(name="rhsp", bufs=16) as rhsp, \
         tc.tile_pool(name="ystp", bufs=6) as ystp, \
         tc.tile_pool(name="scops", bufs=2, space="PSUM") as scops, \
         tc.tile_pool(name="oqps", bufs=1, space="PSUM") as oqps, \
         tc.tile_pool(name="mmps", bufs=2, space="PSUM") as mmps:

        # ---------------- static tiles ----------------
        wq = singles.tile([128, NCC, HPC], BF16)
        wk = singles.tile([128, NCC, HPC], BF16)
        wv = singles.tile([128, NCC, HPC], BF16)
        masks = singles.tile([128, 4, NI], BF16)
        wp = singles.tile([128, NCC, C], BF16)
        ident = singles.tile([128, 128], BF16)
        make_identity(nc, ident)

        # weight stream rides the Act queue, overlapping the SP x stream
        nc.scalar.dma_start(out=wq, in_=wq_h.rearrange("(n p) m -> p n m", p=128))
        nc.scalar.dma_start(out=wk, in_=wk_h.rearrange("(n p) m -> p n m", p=128))
        nc.scalar.dma_start(out=wv, in_=wv_h.rearrange("(n p) m -> p n m", p=128))
        nc.scalar.dma_start(out=masks, in_=mk_h.rearrange("d p i -> p d i"))
        wpsrc = wp_h.rearrange("(n p) m -> p n m", p=128)
        nc.scalar.dma_start(out=wp[:, 0:4, :], in_=wpsrc[:, 0:4, :])
        nc.scalar.dma_start(out=wp[:, 4:8, :], in_=wpsrc[:, 4:8, :])

        # per-batch activations
        qt = [qkp.tile([128, T], BF16, tag="qt", name=f"qt{b}") for b in range(B)]
        kt = [qkp.tile([128, T], BF16, tag="kt", name=f"kt{b}") for b in range(B)]
        vaug = [vaugp.tile([128, NJT, 2, 65], BF16, name=f"vaug{b}")
                for b in range(B)]
        xts: dict = {}

        # ---------------- emission bookkeeping ----------------
        est = {"pe": 0.0, "act": 0.0}
        state = {"last_exp": None, "last_attv": None, "last_norm": None,
                 "exp03": None}
        filler = []   # ordered list of (key, closure)
        fill_pos = [0]

        def pump_one():
            if fill_pos[0] < len(filler):
                key, fn = filler[fill_pos[0]]
                fill_pos[0] += 1
                fn()
                return True
            return False

        def drain_through(key):
            while fill_pos[0] < len(filler):
                k, fn = filler[fill_pos[0]]
                fill_pos[0] += 1
                fn()
                if k == key:
                    break

        def drain_all():
            while pump_one():
                pass

        def pace():
            # keep ~1.2us of PE backlog over the Act (exp) stream
            while est["pe"] < est["act"] + 1200 and pump_one():
                pass

        # ---------------- QKV ----------------
        emitted_parts = set()

        def load_xt(b, ts, half):
            t0 = ts * TSW + half * XLW
            xtile = xtp.tile([128, NCC, XLW], BF16, tag="xt",
                             name=f"xt{b}_{ts}_{half}")
            xsrc = xT_h[b].rearrange("(n p) t -> p n t", p=128)
            nc.sync.dma_start(out=xtile, in_=xsrc[:, :, t0:t0 + XLW])
            xts[(b, ts, half)] = xtile

        def emit_qkv_part(b, ts, which):
            """which: 0=q, 1=k, 2=v token-major"""
            emitted_parts.add((b, ts, which))
            t0 = ts * TSW
            w_t = (wq, wk, wv)[which]
            if which < 2:
                dst = (qt, kt)[which][b]
                ps = mmps.tile([128, TSW], F32, tag="mm", name=f"qk{b}{ts}{which}")
                for hf in range(2):
                    xtile = xts[(b, ts, hf)]
                    for cc in range(NCC):
                        nc.tensor.matmul(ps[:, hf * XLW:(hf + 1) * XLW],
                                         w_t[:, cc, :], xtile[:, cc, :],
                                         start=(cc == 0), stop=(cc == NCC - 1))
                est["pe"] += 16 * 112
                nc.vector.tensor_copy(dst[:, t0:t0 + TSW], ps)
            else:
                # v token-major directly: out[tok, (h, dh)] with lhsT = x
                ps = mmps.tile([128, 4, 2, 64], F32, tag="mm", name=f"v{b}{ts}")
                for q in range(TSW // NJ):
                    hf, ql = divmod(q, 2)
                    xtile = xts[(b, ts, hf)]
                    for cc in range(NCC):
                        nc.tensor.matmul(
                            ps[:, q, :, :],
                            xtile[:, cc, ql * NJ:(ql + 1) * NJ],
                            w_t[:, cc, :],
                            start=(cc == 0), stop=(cc == NCC - 1))
                est["pe"] += 32 * 57
                for q in range(TSW // NJ):
                    jt = ts * (TSW // NJ) + q
                    nc.vector.tensor_copy(vaug[b][:, jt, :, 0:64], ps[:, q])

        # ---------------- attention strip ----------------
        def emit_attv(b, oq, jt, st):
            for h in range(2):
                for qc in range(4):
                    if jt > 4 * st + qc:
                        continue  # key tile entirely above the diagonal
                    state["last_attv"] = nc.tensor.matmul(
                        oq[:, h, qc, 0:65],
                        _wei[jt][:, h, qc * 128:(qc + 1) * 128],
                        vaug[b][:, jt, h, :],
                        start=False, stop=False,
                        skip_group_check=True,
                    )
                    est["pe"] += 65 * 0.43 + 8

        _wei = {}

        def emit_strip(b, st, after_first=None):
            i0 = st * NI
            njt = 4 * (st + 1)
            oq = oqps.tile([128, 2, 4, 128], F32, tag="oq", name=f"oq{b}{st}")
            # accumulate via memset + start=False matmuls: 8 concurrent
            # (h,qt) groups share two psum banks, which start_tensor_calc's
            # bank-wide pending-zero cannot express
            nc.vector.memset(oq[:, :, :, 0:65], 0.0)
            _wei.clear()
            for jt in range(njt):
                j0 = jt * NJ
                d = jt - (njt - 4)
                lo = max(d, 0) * 128
                sco = scops.tile([128, 2, NI], F32, tag="sco", name=f"s{b}{st}{jt}")
                for h in range(2):
                    nc.tensor.matmul(
                        sco[:, h, lo:],
                        kt[b][h * 64:(h + 1) * 64, j0:j0 + NJ],
                        qt[b][h * 64:(h + 1) * 64, i0 + lo:i0 + NI],
                        start=True, stop=True,
                    )
                est["pe"] += 2 * (NI - lo) * 0.43 + 10
                wei = weip.tile([128, 2, NI], BF16, tag="wei", name=f"w{b}{st}{jt}")
                _wei[jt] = wei
                if d < 1:
                    state["last_exp"] = nc.scalar.activation(
                        wei, sco, mybir.ActivationFunctionType.Exp,
                        scale=SCALE)
                    est["act"] += 1038
                else:
                    state["last_exp"] = nc.scalar.activation(
                        wei[:, :, lo:], sco[:, :, lo:],
                        mybir.ActivationFunctionType.Exp, scale=SCALE)
                    est["act"] += 2 * (NI - lo) * 0.833 + 185
                if d >= 0:
                    for h in range(2):
                        nc.vector.tensor_mul(
                            wei[:, h, lo:], wei[:, h, lo:], masks[:, d, lo:])
                if jt == 0 and after_first is not None:
                    after_first()
                # att@v for the PREVIOUS tile: keeps the PE one scores-tile
                # ahead of the exp stream
                if jt > 0:
                    emit_attv(b, oq, jt - 1, st)
                pace()
            emit_attv(b, oq, njt - 1, st)

            # ---- normalize and ship to reshard slots in [token, head_dim]
            rcp = rcpp.tile([128, 2, 4], F32, tag="rcp", name=f"r{b}{st}")
            att_big = attnp.tile([128, 4, 128], BF16, tag="attn",
                                 name=f"an{b}{st}")
            for h in range(2):
                nc.vector.reciprocal(rcp[:, h, :], oq[:, h, :, 64])
            for qc in range(4):
                for h in range(2):
                    state["last_norm"] = nc.vector.tensor_scalar_mul(
                        att_big[:, qc, h * 64:(h + 1) * 64],
                        oq[:, h, qc, 0:64],
                        rcp[:, h, qc:qc + 1],
                    )
                if (b, st) == (1, 3):
                    eng = nc.sync if qc % 2 == 0 else nc.scalar
                    eng.dma_start(out=a3_in[4 * b + qc],
                                  in_=att_big[:, qc, :])
            if st < 2:
                s0 = 4 * b + 2 * st
                nc.sync.dma_start(
                    out=a1_in[s0:s0 + 2].rearrange("s (two p) h -> p (s two) h",
                                                   p=128),
                    in_=att_big)
            elif (b, st) != (1, 3):
                dst = a2_in if st == 2 else a3_in
                nc.sync.dma_start(
                    out=dst[4 * b:4 * b + 4].rearrange("s t h -> t s h"),
                    in_=att_big)
            if (b, st) == (0, 3):
                state["exp03"] = state["last_exp"]

        # ---------------- output projection ----------------
        def emit_proj(a_out, width, ycol0):
            two = width // 128
            reads = []
            for j in range(NCORES):
                rT = rhsp.tile([128, two, 128], BF16, tag="rT",
                               name=f"rT{ycol0}_{j}")
                eng = nc.scalar if j % 2 == 0 else nc.sync
                d = eng.dma_start(
                    out=rT,
                    in_=a_out[j].rearrange("(two p) h -> p two h", p=128))
                # anti-hoist: keep tail reads behind the exp stream so the
                # scheduling pass cannot move them (and their collective
                # waits) into the middle of the strip phase
                anchor = state["exp03"] if ycol0 == 0 else state["last_exp"]
                add_dep_helper(d.ins, anchor.ins)
                reads.append(rT)
            rhs = []
            for j in range(NCORES):
                tp = mmps.tile([128, two, 128], BF16, tag="mm",
                               name=f"tp{ycol0}_{j}")
                for i in range(two):
                    t_ = nc.tensor.transpose(tp[:, i, :], reads[j][:, i, :],
                                             ident)
                    add_dep_helper(t_.ins, state["last_attv"].ins)
                r = rhsp.tile([128, width], BF16, tag="rhs",
                              name=f"rhs{ycol0}_{j}")
                c_ = nc.vector.tensor_copy(r, tp)
                add_dep_helper(c_.ins, state["last_norm"].ins)
                rhs.append(r)
            for nt in range(8):
                py = mmps.tile([128, width], F32, tag="mm", name=f"py{ycol0}{nt}")
                for j in range(NCORES):
                    nc.tensor.matmul(py, wp[:, j, nt * 128:(nt + 1) * 128],
                                     rhs[j],
                                     start=(j == 0), stop=(j == NCORES - 1))

                est["pe"] += 8 * width * 0.43 + 10
                yo = ystp.tile([128, width], F32, tag="yst", name=f"y{ycol0}{nt}")
                nc.vector.tensor_copy(yo, py)
                eng = nc.sync if nt % 2 == 0 else nc.scalar
                eng.dma_start(
                    out=y_h[nt * 128:(nt + 1) * 128, ycol0:ycol0 + width],
                    in_=yo)

        def emit_proj_tail(a_out, ycol0):
            # j-outer accumulation into the freed oq psum banks: matmuls for
            # slot j start as soon as its read+transpose land, instead of
            # waiting for all eight slots
            py = oqps.tile([128, 2, 4, 128], F32, tag="oq", name="py3")
            nc.vector.memset(py, 0.0)
            for j in range(NCORES):
                # C3 is the last collective, so the DMA xbar is free: a
                # transposing read lands [hd, tok] directly in SBUF
                rj = rhsp.tile([128, 128], BF16, tag="rhs", name=f"rh3_{j}")
                eng = nc.sync if j % 2 == 0 else nc.scalar
                d = eng.dma_start(out=rj, in_=a_out[j], transpose=True)
                add_dep_helper(d.ins, state["last_exp"].ins)
                for nt in range(8):
                    nc.tensor.matmul(py[:, nt // 4, nt % 4, :],
                                     wp[:, j, nt * 128:(nt + 1) * 128],
                                     rj,
                                     start=False, stop=False,
                                     skip_group_check=True)
            est["pe"] += 64 * 57
            yo = ystp.tile([128, 8, 128], F32, tag="yst3", name="y3st", bufs=1)
            nc.vector.tensor_copy(yo[:, 0:4, :], py[:, 0])
            nc.scalar.copy(yo[:, 4:8, :], py[:, 1])
            ydst = y_h.rearrange("(n p) m -> p n m", p=128)
            nc.sync.dma_start(out=ydst[:, 0:4, ycol0:ycol0 + 128], in_=yo[:, 0:4, :])
            nc.scalar.dma_start(out=ydst[:, 4:8, ycol0:ycol0 + 128], in_=yo[:, 4:8, :])

        # ================= emission order =================
        # SP queue: x chunks for b0 lead, b1 trickles in later
        for ts in range(4):
            load_xt(0, ts, 0)
            load_xt(0, ts, 1)
        for b in range(B):
            nc.vector.memset(vaug[b], 1.0)

        emit_qkv_part(0, 0, 0)
        emit_qkv_part(0, 0, 1)
        emit_qkv_part(0, 0, 2)

        def qkv_unit(b, ts, which):
            def go():
                if (b, ts, which) not in emitted_parts:
                    emit_qkv_part(b, ts, which)
            return go
        for bb, ts in ((0, 1), (1, 0), (1, 1)):
            for which in range(3):
                filler.append(((bb, ts, which), qkv_unit(bb, ts, which)))

        load_xt(1, 0, 0)
        load_xt(1, 0, 1)

        emit_strip(0, 0)
        load_xt(1, 1, 0)
        load_xt(1, 1, 1)
        drain_through((0, 1, 1))
        emit_strip(0, 1, after_first=lambda: emit_qkv_part(0, 1, 2))
        load_xt(1, 2, 0)
        load_xt(1, 2, 1)
        drain_through((1, 0, 2))
        emit_strip(1, 0)
        load_xt(1, 3, 0)
        load_xt(1, 3, 1)
        drain_through((1, 1, 1))
        emit_strip(1, 1, after_first=lambda: emit_qkv_part(1, 1, 2))

        for bb, ts in ((0, 2), (1, 2)):
            for which in range(3):
                filler.append(((bb, ts, which), qkv_unit(bb, ts, which)))
        drain_through((0, 2, 1))
        emit_strip(0, 2, after_first=lambda: emit_qkv_part(0, 2, 2))
        drain_through((1, 2, 1))
        emit_strip(1, 2, after_first=lambda: emit_qkv_part(1, 2, 2))

        for bb, ts in ((0, 3), (1, 3)):
            for which in range(3):
                filler.append(((bb, ts, which), qkv_unit(bb, ts, which)))
        drain_through((0, 3, 1))
        emit_strip(0, 3, after_first=lambda: emit_qkv_part(0, 3, 2))
        drain_through((1, 3, 1))
        emit_strip(1, 3, after_first=lambda: emit_qkv_part(1, 3, 2))
        drain_all()

        # ---- collectives last: the Pool queue carries nothing else, so
        # each launches the moment its input slots land. Receive-side
        # transposes only serialize against their own (true) collective.
        nc.gpsimd.collective_compute(
            "AllToAll", mybir.AluOpType.bypass,
            replica_groups=[list(range(NCORES))],
            ins=[a1_in[:, :, :].opt()], outs=[a1_out[:, :, :].opt()],
        )
        emit_proj(a1_out, 256, 0)
        nc.gpsimd.collective_compute(
            "AllToAll", mybir.AluOpType.bypass,
            replica_groups=[list(range(NCORES))],
            ins=[a2_in[:, :, :].opt()], outs=[a2_out[:, :, :].opt()],
        )
        emit_proj(a2_out, 128, 256)
        nc.gpsimd.collective_compute(
            "AllToAll", mybir.AluOpType.bypass,
            replica_groups=[list(range(NCORES))],
            ins=[a3_in[:, :, :].opt()], outs=[a3_out[:, :, :].opt()],
        )
        emit_proj_tail(a3_out, 384)
    return nc


_NC_CACHE = {}


def _get_nc():
    if "nc" not in _NC_CACHE:
        _NC_CACHE["nc"] = build_nc()
    return _NC_CACHE["nc"]


def _host_masks():
    jl = np.arange(NJ)[:, None]
    il = np.arange(NI)[None, :]
    return np.stack([(il >= jl + d * 128) for d in range(4)]).astype(ml_dtypes.bfloat16)


def kernel(x, Wk, Wq, Wv, Wp, bp):
    x = np.asarray(x)
    xT = np.ascontiguousarray(x.transpose(0, 2, 1)).astype(ml_dtypes.bfloat16)
    wpb = np.asarray(Wp).astype(ml_dtypes.bfloat16)
    masks = _host_masks()
    in_maps = []
    for c in range(NCORES):
        cs = slice(c * HPC, (c + 1) * HPC)
        in_maps.append({
            "xT": xT,
            "wq": np.ascontiguousarray(Wq[:, cs]).astype(ml_dtypes.bfloat16),
            "wk": np.ascontiguousarray(Wk[:, cs]).astype(ml_dtypes.bfloat16),
            "wv": np.ascontiguousarray(Wv[:, cs]).astype(ml_dtypes.bfloat16),
            "wp": wpb,
            "masks": masks,
        })
    res = run_bass_kernel_spmd(_get_nc(), in_maps, list(range(NCORES)))
    # y_out cols: [0:256) chunk1, [256:384) chunk2, [384:512) chunk3
    yT = np.zeros((B, T, C), np.float32)
    for c in range(NCORES):
        yo = res.results[c]["y_out"]
        b, q = divmod(c, 4)
        yT[b, 256 * q:256 * (q + 1), :] = yo[:, 0:256].T
        yT[b, 1024 + 128 * q:1024 + 128 * (q + 1), :] = yo[:, 256:384].T
        yT[b, 1536 + 128 * q:1536 + 128 * (q + 1), :] = yo[:, 384:512].T
    y = yT + np.asarray(bp)[None, None, :]
    return np.ascontiguousarray(y, dtype=np.float32)
